# revision 14
# baseline (speedup 1.0000x reference)
"""TRN2 Bass kernel for nn_Attention_188978561266.

Reference computation (b=4, s=1024, d=1024, 16 heads x 64):
    qkv = x @ Wqkv ; split q,k,v
    q = q / (sqrt(mean(q^2 over ALL elements)) + eps) * scale_q   (global scalar RMS)
    k = k / (sqrt(mean(k^2 over ALL elements)) + eps) * scale_k
    attn = softmax(q @ k^T)  (no 1/sqrt(d_head), no mask)
    out = (attn @ v) @ Wo

Sharding: 8 cores = (batch b in 0..3) x (head-half in 0..1). Each core computes
qkv for its batch restricted to its 8 heads (tensor-parallel QKV columns),
full attention for those heads, and a partial output projection. Host sums
the two partial outputs per batch. The global RMS needs a cross-core
AllReduce of two scalars (sum q^2, sum k^2).

All matmuls run in float32r (full PE rate, 11-bit mantissa RNE — measured
bit-exact vs host emulation), accumulating in fp32 PSUM. The per-d_head
scale_q*scale_k vector folds into Q at the psum->SBUF copy; the runtime
1/((rms_q+eps)(rms_k+eps)) scalar folds into the softmax exp's scale operand,
so the PE never waits on the collective for the S = K^T Q matmuls.
"""

import sys

sys.path.insert(0, "/opt/trn_rl_repo")

import numpy as np

import concourse.bacc as bacc
import concourse.mybir as mybir
from concourse import library_config, tile
from concourse.bass_utils import run_bass_kernel_spmd

F32 = mybir.dt.float32
F32R = mybir.dt.float32r
AF = mybir.ActivationFunctionType
ALU = mybir.AluOpType
AX = mybir.AxisListType

P = 128
D = 1024
S = 1024
N_HEAD = 16
DH = 64
NHL = 8          # heads per core
DC = 8           # d contraction chunks of 128
EPS = 1e-6
COUNT = 4 * 1024 * 1024   # elements of the full q (or k) tensor
import os as _os
N_CORES = int(_os.environ.get("KN_CORES", "8"))
REPLICAS = [list(range(N_CORES))]

_CACHE = {}


def _rne11(x: np.ndarray) -> np.ndarray:
    """Round float32 to 11 explicit mantissa bits (matches HW float32r)."""
    u = np.ascontiguousarray(x, dtype=np.float32).view(np.uint32).astype(np.uint64)
    shift = 12
    bias = ((u >> shift) & 1) + ((1 << (shift - 1)) - 1)
    return (((u + bias) >> shift) << shift).astype(np.uint32).view(np.float32)


def _build():
    nc = bacc.Bacc("TRN2", target_bir_lowering=False, debug=False, num_devices=N_CORES)

    xt = nc.dram_tensor("xt", [P, DC, S], F32R, kind="ExternalInput")
    wqk = nc.dram_tensor("wqk", [P, 8, DC, P], F32R, kind="ExternalInput")
    wv = nc.dram_tensor("wv", [P, DC, NHL * DH], F32R, kind="ExternalInput")
    wo = nc.dram_tensor("wo", [P, 4, D], F32R, kind="ExternalInput")
    qscale = nc.dram_tensor("qscale", [P, 4], F32, kind="ExternalInput")
    zpart = nc.dram_tensor("zpart", [S, D], F32, kind="ExternalOutput")

    with tile.TileContext(nc) as tc:
        with (
            tc.tile_pool(name="big", bufs=1) as big,
            tc.tile_pool(name="wq", bufs=3) as wqp,
            tc.tile_pool(name="ep", bufs=3) as ep,
            tc.tile_pool(name="scr", bufs=2) as scrp,
            tc.tile_pool(name="small", bufs=2) as smallp,
            tc.tile_pool(name="stats", bufs=1) as stp,
            tc.tile_pool(name="ps", bufs=3, space="PSUM") as psp,
            tc.tile_pool(name="pav", bufs=2, space="PSUM") as pav,
            tc.tile_pool(name="dram", bufs=1, space="DRAM") as dramp,
        ):
            # ---- persistent SBUF tensors ----
            xT = big.tile([P, DC, S], F32R, tag="xT")
            QT = big.tile([P, 4, S], F32R, tag="QT")
            KT = big.tile([P, 4, S], F32R, tag="KT")
            Vt = big.tile([P, 8, NHL, DH + 1], F32R, tag="Vt")
            zT = big.tile([P, 4, S], F32R, tag="zT")

            Wo_sb = big.tile([P, 4, D], F32R, tag="Wo")

            qs_sb = stp.tile([P, 4], F32, tag="qs")
            sq_acc = stp.tile([P, 8], F32, tag="sqacc")
            qk2 = stp.tile([P, 2], F32, tag="qk2")
            g_sb = stp.tile([2, 1], F32, tag="gsb")
            gsum = stp.tile([1, 2], F32, tag="gsum")
            sc_a = stp.tile([1, 2], F32, tag="sca")
            sc_b = stp.tile([1, 2], F32, tag="scb")
            sc_c = stp.tile([1, 2], F32, tag="scc")
            pm = stp.tile([1, 1], F32, tag="pm")
            cinv = stp.tile([1, 1], F32, tag="cinv")
            c_bc = stp.tile([P, 1], F32, tag="cbc")

            dsq = stp.tile([1, 2], F32, tag="dsq")
            ones_row = stp.tile([1, P], F32, tag="ones_row")
            ones_col = stp.tile([P, 1], F32, tag="ones_col")
            ones64r = stp.tile([1, DH], F32R, tag="ones64r")
            ones_blk = stp.tile([P, 8, NHL, 1], F32, tag="ones_blk")

            nc.gpsimd.load_library(library_config.attn)
            cc_warm_in = dramp.tile([2, 1], F32, tag="ccwi")
            cc_warm_out = dramp.tile([2, 1], F32, tag="ccwo", addr_space="Shared")
            nc.gpsimd.collective_compute(
                "AllReduce",
                ALU.add,
                replica_groups=REPLICAS,
                ins=[cc_warm_in[:]],
                outs=[cc_warm_out[:]],
            )
            # ---- input DMAs + consts (first weight tiles before bulk x) ----
            wts = {}
            wts[0] = wqp.tile([P, DC, P], F32R, tag="wq", name="wt0")
            for dc in range(DC):
                nc.sync.dma_start(wts[0][:, dc, :], wqk[:, 0, dc, :])
            nc.sync.dma_start(xT[:, 0, 0:512], xt[:, 0, 0:512])
            nc.sync.dma_start(xT[:, 0, 512:1024], xt[:, 0, 512:1024])
            wts[1] = wqp.tile([P, DC, P], F32R, tag="wq", name="wt1")
            nc.sync.dma_start(wts[1][:], wqk[:, 1, :, :])
            for dc in range(1, DC):
                nc.sync.dma_start(xT[:, dc, :], xt[:, dc, :])
            nc.sync.dma_start(qs_sb[:], qscale[:])
            nc.vector.memset(ones_row[:], 1.0)
            nc.vector.memset(ones_col[:], 1.0)
            nc.vector.memset(ones_blk[:], 1.0)
            nc.vector.tensor_copy(ones64r[:], ones_row[:, 0:DH])
            nc.vector.tensor_copy(Vt[:, :, :, DH : DH + 1], ones_blk[:])

            # ---- phase A: q,k projections (transposed layout) + raw sum-sq ----
            for ct in range(8):
                if ct in wts:
                    wt = wts.pop(ct)
                else:
                    wt = wqp.tile([P, DC, P], F32R, tag="wq")
                    nc.sync.dma_start(wt[:], wqk[:, ct, :, :])
                ps = psp.tile([P, 2, 512], F32, tag="mm2")
                for st in range(2):
                    for dc in range(DC):
                        nc.tensor.matmul(
                            ps[:, st, :],
                            lhsT=wt[:, dc, :],
                            rhs=xT[:, dc, st * 512 : (st + 1) * 512],
                            start=(dc == 0),
                            stop=(dc == DC - 1),
                        )
                scr = scrp.tile([P, 2, 512], F32, tag="scr")
                nc.scalar.activation(
                    scr[:], ps[:], AF.Square, accum_out=sq_acc[:, ct : ct + 1]
                )
                flat = ps[:].rearrange("p a b -> p (a b)")
                if ct < 4:
                    nc.vector.tensor_scalar(
                        QT[:, ct, :], flat, qs_sb[:, ct : ct + 1], None, ALU.mult
                    )
                else:
                    nc.vector.tensor_copy(KT[:, ct - 4, :], flat)

            # prefetch the sqrt ACT table set while PE is still busy
            nc.scalar.activation(dsq[:], ones_row[:, 0:2], AF.Sqrt)

            # ---- global RMS part 1: reduce + AllReduce + scalar chain ----
            nc.vector.reduce_sum(qk2[:, 0:1], sq_acc[:, 0:4], axis=AX.X)
            nc.vector.reduce_sum(qk2[:, 1:2], sq_acc[:, 4:8], axis=AX.X)
            g_ps = pav.tile([P, 512], F32, tag="av", name="g_ps")
            nc.tensor.matmul(
                g_ps[0:2, 0:1], lhsT=qk2[:], rhs=ones_col[:], start=True, stop=True
            )
            nc.vector.tensor_copy(g_sb[:], g_ps[0:2, 0:1])
            cc_in = dramp.tile([2, 1], F32, tag="ccin")
            cc_out = dramp.tile([2, 1], F32, tag="ccout", addr_space="Shared")
            nc.sync.dma_start(cc_in[:], g_sb[:])
            nc.gpsimd.collective_compute(
                "AllReduce",
                ALU.add,
                replica_groups=REPLICAS,
                ins=[cc_in[:]],
                outs=[cc_out[:]],
            )
            nc.sync.dma_start(gsum[:], cc_out[:].rearrange("a b -> b a"))
            # mean, sqrt (+1 Newton step), +eps, product, reciprocal
            nc.vector.tensor_scalar_mul(sc_a[:], gsum[:], 1.0 / COUNT)  # m
            nc.scalar.activation(sc_b[:], sc_a[:], AF.Sqrt)             # r0
            nc.vector.reciprocal(sc_c[:], sc_b[:])                      # 1/r
            nc.vector.tensor_mul(sc_c[:], sc_a[:], sc_c[:])             # m/r
            nc.vector.tensor_add(sc_b[:], sc_b[:], sc_c[:])             # r + m/r
            nc.vector.tensor_scalar(sc_b[:], sc_b[:], 0.5, EPS, ALU.mult, ALU.add)
            nc.scalar.activation(dsq[:], ones_row[:, 0:2], AF.Exp)
            nc.vector.tensor_mul(pm[:], sc_b[:, 0:1], sc_b[:, 1:2])
            nc.vector.reciprocal(cinv[:], pm[:])

            # ---- phase A2: v projection (covers the collective latency) ----
            Wv_sb = ep.tile([P, DC, NHL * DH], F32R, tag="E", name="Wv_sb")
            nc.sync.dma_start(Wv_sb[:], wv[:])
            for u in range(4):
                ps = psp.tile([P, 2, 512], F32, tag="mm2")
                for j in range(2):
                    sm = 2 * u + j
                    for dc in range(DC):
                        nc.tensor.matmul(
                            ps[:, j, :],
                            lhsT=xT[:, dc, sm * P : (sm + 1) * P],
                            rhs=Wv_sb[:, dc, :],
                            start=(dc == 0),
                            stop=(dc == DC - 1),
                        )
                nc.vector.tensor_copy(
                    Vt[:, 2 * u : 2 * u + 2, :, 0:DH],
                    ps[:].rearrange("p a (h d) -> p a h d", h=NHL),
                )

            # ---- global RMS part 2: broadcast scalar to 128 partitions ----
            nc.gpsimd.partition_broadcast(c_bc[:], cinv[:])
            nc.sync.dma_start(Wo_sb[:], wo[:])

            # ---- PE warm-keepers: cover the AllReduce+chain stall so the
            # HAM clock gate stays at 8/8 entering the attention phase ----
            warm_ps = pav.tile([P, 512], F32, tag="av", name="warm_ps")
            for _ in range(70):
                nc.tensor.matmul(
                    warm_ps[:],
                    lhsT=KT[:, 0, 0:P],
                    rhs=KT[:, 0, 0:512],
                    start=True,
                    stop=True,
                )

            # ---- phase C: attention, head pairs packed on PE row groups ----
            # heads (2g, 2g+1) live at partitions 0:64 / 64:128 of chunk g;
            # alternating their S^T matmuls engages concurrent 64-row tiles.
            for g in (2, 3, 0, 1):
                h0 = 2 * g
                for t in range(2):
                    tsl = slice(t * 512, (t + 1) * 512)
                    E_pair = [
                        ep.tile([P, 8, 512], F32R, tag="E", name=f"E_{g}_{t}_{i}")
                        for i in range(2)
                    ]
                    ps_pair = {}
                    for j in range(4):
                        for i in range(2):
                            ps_pair[(j, i)] = psp.tile(
                                [P, 2, 512], F32, tag="mm2", name=f"s_{j}_{i}"
                            )
                        for jj in range(2):
                            skt = 2 * j + jj
                            for i in range(2):
                                hp = i * DH
                                nc.tensor.matmul(
                                    ps_pair[(j, i)][:, jj, :],
                                    lhsT=KT[hp : hp + DH, g, skt * P : (skt + 1) * P],
                                    rhs=QT[hp : hp + DH, g, tsl],
                                    start=True,
                                    stop=True,
                                )
                        for i in range(2):
                            nc.scalar.activation(
                                E_pair[i][:, 2 * j : 2 * j + 2, :],
                                ps_pair[(j, i)][:],
                                AF.Exp,
                                scale=c_bc[:, 0:1],
                            )
                    for i in range(2):
                        h = h0 + i
                        E_t = E_pair[i]
                        ps_av = pav.tile([P, 512], F32, tag="av", name=f"av_{i}")
                        for skc in range(8):
                            nc.tensor.matmul(
                                ps_av[0 : DH + 1, :],
                                lhsT=Vt[:, skc, h, :],
                                rhs=E_t[:, skc, :],
                                start=(skc == 0),
                                stop=(skc == 7),
                            )
                        zun = smallp.tile([DH + 1, 512], F32, tag="zun", name=f"zun_{i}")
                        nc.vector.tensor_copy(zun[:], ps_av[0 : DH + 1, :])
                        rs0 = smallp.tile([1, 512], F32, tag="rs0", name=f"rs0_{i}")
                        nc.vector.tensor_copy(rs0[:], zun[DH : DH + 1, :])
                        rs_r = smallp.tile([1, 512], F32, tag="rs", name=f"rs_{i}")
                        nc.vector.reciprocal_approx_fast(rs_r[:], rs0[:])
                        bc_sb = smallp.tile([DH, 512], F32, tag="bcs", name=f"bc_{i}")
                        nc.gpsimd.partition_broadcast(bc_sb[:], rs_r[:])
                        if h < 4:
                            nc.vector.tensor_mul(
                                zT[0:DH, h, tsl], zun[0:DH, :], bc_sb[:]
                            )
                        else:
                            ztmp = smallp.tile(
                                [DH, 512], F32R, tag="ztmp", name=f"zt_{i}"
                            )
                            nc.vector.tensor_mul(ztmp[:], zun[0:DH, :], bc_sb[:])
                            nc.sync.dma_start(zT[DH:P, h - 4, tsl], ztmp[:])

            # ---- phase D: output projection (partial over local heads) ----
            for sm in range(8):
                ps_o = psp.tile([P, 2, 512], F32, tag="mm2")
                for nt in range(2):
                    for hdc in range(4):
                        nc.tensor.matmul(
                            ps_o[:, nt, :],
                            lhsT=zT[:, hdc, sm * P : (sm + 1) * P],
                            rhs=Wo_sb[:, hdc, nt * 512 : (nt + 1) * 512],
                            start=(hdc == 0),
                            stop=(hdc == 3),
                        )
                ob = scrp.tile([P, 2, 512], F32, tag="scr")
                nc.any.tensor_copy(ob[:], ps_o[:])
                nc.sync.dma_start(
                    zpart[sm * P : (sm + 1) * P, :],
                    ob[:].rearrange("p a b -> p (a b)"),
                )

    nc.compile()
    return nc


def _get_nc():
    if "nc" not in _CACHE:
        _CACHE["nc"] = _build()
    return _CACHE["nc"]


def _prep_core_inputs(x, Wqkv, Wo, scale_q, scale_k):
    """Host-side shard + layout prep. Returns list of 8 in_maps."""
    x = np.asarray(x, dtype=np.float32)
    Wqkv = np.asarray(Wqkv, dtype=np.float32)
    Wo = np.asarray(Wo, dtype=np.float32)
    scale_q = np.asarray(scale_q, dtype=np.float32)
    scale_k = np.asarray(scale_k, dtype=np.float32)

    # combined per-d_head scale folded into Q (applied after raw sum-sq)
    qs_vec = np.tile(scale_q * scale_k, NHL)          # [512]
    qs_dev = np.ascontiguousarray(qs_vec.reshape(4, P).T)  # [128,4]

    xt_all = []
    for b in range(4):
        xTb = _rne11(x[b].T)                           # [d, s]
        xt_all.append(
            np.ascontiguousarray(xTb.reshape(DC, P, S).transpose(1, 0, 2))
        )  # [128, 8, 1024]

    in_maps = []
    for c in range(8):
        b = c // 2
        hh = (c % 2) * NHL
        cols = slice(hh * DH, (hh + NHL) * DH)
        wq_c = Wqkv[:, 0 * D:1 * D][:, cols]           # [1024, 512]
        wk_c = Wqkv[:, 1 * D:2 * D][:, cols]
        wv_c = Wqkv[:, 2 * D:3 * D][:, cols]
        wqk_c = _rne11(np.concatenate([wq_c, wk_c], axis=1))  # [1024, 1024]
        # [p, ct, dc, n]: per-ct slices are contiguous 4KB-per-partition DMAs
        wqk_dev = np.ascontiguousarray(
            wqk_c.reshape(DC, P, 8, P).transpose(1, 2, 0, 3)
        )
        wv_dev = np.ascontiguousarray(
            _rne11(wv_c).reshape(DC, P, NHL * DH).transpose(1, 0, 2)
        )
        # Wo rows for local heads, arranged [128, 4, 1024]:
        # chunk k low half = head k, high half = head 4+k
        wo_loc = _rne11(Wo[(hh * DH):(hh + NHL) * DH, :])   # [512, 1024]
        wo_dev = np.empty((P, 4, D), dtype=np.float32)
        for k in range(4):
            wo_dev[0:DH, k, :] = wo_loc[k * DH:(k + 1) * DH, :]
            wo_dev[DH:P, k, :] = wo_loc[(4 + k) * DH:(5 + k) * DH, :]
        in_maps.append(
            {
                "xt": xt_all[b],
                "wqk": wqk_dev,
                "wv": wv_dev,
                "wo": np.ascontiguousarray(wo_dev),
                "qscale": qs_dev,
            }
        )
    return in_maps


def run(x, Wqkv, Wo, scale_q, scale_k, trace=False):
    nc = _get_nc()
    in_maps = _prep_core_inputs(x, Wqkv, Wo, scale_q, scale_k)
    res = run_bass_kernel_spmd(
        nc, in_maps[:N_CORES], core_ids=list(range(N_CORES)), trace=trace
    )
    out = np.empty((4, S, D), dtype=np.float32)
    for b in range(4):
        if N_CORES == 8:
            out[b] = res.results[2 * b]["zpart"] + res.results[2 * b + 1]["zpart"]
    return out, res


def kernel(x, Wqkv, Wo, scale_q, scale_k):
    out, _ = run(x, Wqkv, Wo, scale_q, scale_k, trace=False)
    return out


# revision 15
# speedup vs baseline: 1.0097x; 1.0097x over previous
"""TRN2 Bass kernel for nn_Attention_188978561266.

Reference computation (b=4, s=1024, d=1024, 16 heads x 64):
    qkv = x @ Wqkv ; split q,k,v
    q = q / (sqrt(mean(q^2 over ALL elements)) + eps) * scale_q   (global scalar RMS)
    k = k / (sqrt(mean(k^2 over ALL elements)) + eps) * scale_k
    attn = softmax(q @ k^T)  (no 1/sqrt(d_head), no mask)
    out = (attn @ v) @ Wo

Sharding: 8 cores = (batch b in 0..3) x (head-half in 0..1). Each core computes
qkv for its batch restricted to its 8 heads (tensor-parallel QKV columns),
full attention for those heads, and a partial output projection. Host sums
the two partial outputs per batch. The global RMS needs a cross-core
AllReduce of two scalars (sum q^2, sum k^2).

All matmuls run in float32r (full PE rate, 11-bit mantissa RNE — measured
bit-exact vs host emulation), accumulating in fp32 PSUM. The per-d_head
scale_q*scale_k vector folds into Q at the psum->SBUF copy; the runtime
1/((rms_q+eps)(rms_k+eps)) scalar folds into the softmax exp's scale operand,
so the PE never waits on the collective for the S = K^T Q matmuls.
"""

import sys

sys.path.insert(0, "/opt/trn_rl_repo")

import numpy as np

import concourse.bacc as bacc
import concourse.mybir as mybir
from concourse import library_config, tile
from concourse.bass_utils import run_bass_kernel_spmd

F32 = mybir.dt.float32
F32R = mybir.dt.float32r
AF = mybir.ActivationFunctionType
ALU = mybir.AluOpType
AX = mybir.AxisListType

P = 128
D = 1024
S = 1024
N_HEAD = 16
DH = 64
NHL = 8          # heads per core
DC = 8           # d contraction chunks of 128
EPS = 1e-6
COUNT = 4 * 1024 * 1024   # elements of the full q (or k) tensor
import os as _os
N_CORES = int(_os.environ.get("KN_CORES", "8"))
REPLICAS = [list(range(N_CORES))]

_CACHE = {}


def _rne11(x: np.ndarray) -> np.ndarray:
    """Round float32 to 11 explicit mantissa bits (matches HW float32r)."""
    u = np.ascontiguousarray(x, dtype=np.float32).view(np.uint32).astype(np.uint64)
    shift = 12
    bias = ((u >> shift) & 1) + ((1 << (shift - 1)) - 1)
    return (((u + bias) >> shift) << shift).astype(np.uint32).view(np.float32)


def _build():
    nc = bacc.Bacc("TRN2", target_bir_lowering=False, debug=False, num_devices=N_CORES)

    xt = nc.dram_tensor("xt", [P, DC, S], F32R, kind="ExternalInput")
    wqk = nc.dram_tensor("wqk", [P, 8, DC, P], F32R, kind="ExternalInput")
    wv = nc.dram_tensor("wv", [P, DC, NHL * DH], F32R, kind="ExternalInput")
    wo = nc.dram_tensor("wo", [P, 4, D], F32R, kind="ExternalInput")
    qscale = nc.dram_tensor("qscale", [P, 4], F32, kind="ExternalInput")
    zpart = nc.dram_tensor("zpart", [S, D], F32, kind="ExternalOutput")

    with tile.TileContext(nc) as tc:
        with (
            tc.tile_pool(name="big", bufs=1) as big,
            tc.tile_pool(name="wq", bufs=3) as wqp,
            tc.tile_pool(name="ep", bufs=3) as ep,
            tc.tile_pool(name="scr", bufs=2) as scrp,
            tc.tile_pool(name="small", bufs=2) as smallp,
            tc.tile_pool(name="stats", bufs=1) as stp,
            tc.tile_pool(name="ps", bufs=3, space="PSUM") as psp,
            tc.tile_pool(name="pav", bufs=2, space="PSUM") as pav,
            tc.tile_pool(name="dram", bufs=1, space="DRAM") as dramp,
        ):
            # ---- persistent SBUF tensors ----
            xT = big.tile([P, DC, S], F32R, tag="xT")
            QT = big.tile([P, 4, S], F32R, tag="QT")
            KT = big.tile([P, 4, S], F32R, tag="KT")
            Vt = big.tile([P, 8, NHL, DH + 1], F32R, tag="Vt")
            zT = big.tile([P, 4, S], F32R, tag="zT")

            Wo_sb = big.tile([P, 4, D], F32R, tag="Wo")

            qs_sb = stp.tile([P, 4], F32, tag="qs")
            sq_acc = stp.tile([P, 8], F32, tag="sqacc")
            qk2 = stp.tile([P, 2], F32, tag="qk2")
            g_sb = stp.tile([2, 1], F32, tag="gsb")
            gsum = stp.tile([1, 2], F32, tag="gsum")
            sc_a = stp.tile([1, 2], F32, tag="sca")
            sc_b = stp.tile([1, 2], F32, tag="scb")
            sc_c = stp.tile([1, 2], F32, tag="scc")
            pm = stp.tile([1, 1], F32, tag="pm")
            cinv = stp.tile([1, 1], F32, tag="cinv")
            c_bc = stp.tile([P, 1], F32, tag="cbc")

            dsq = stp.tile([1, 2], F32, tag="dsq")
            ones_row = stp.tile([1, P], F32, tag="ones_row")
            ones_col = stp.tile([P, 1], F32, tag="ones_col")
            ones64r = stp.tile([1, DH], F32R, tag="ones64r")
            ones_blk = stp.tile([P, 8, NHL, 1], F32, tag="ones_blk")

            nc.gpsimd.load_library(library_config.attn)
            cc_warm_in = dramp.tile([2, 1], F32, tag="ccwi")
            cc_warm_out = dramp.tile([2, 1], F32, tag="ccwo", addr_space="Shared")
            nc.gpsimd.collective_compute(
                "AllReduce",
                ALU.add,
                replica_groups=REPLICAS,
                ins=[cc_warm_in[:]],
                outs=[cc_warm_out[:]],
            )
            # ---- input DMAs + consts (first weight tiles before bulk x) ----
            wts = {}
            wts[0] = wqp.tile([P, DC, P], F32R, tag="wq", name="wt0")
            for dc in range(DC):
                nc.sync.dma_start(wts[0][:, dc, :], wqk[:, 0, dc, :])
            nc.sync.dma_start(xT[:, 0, 0:512], xt[:, 0, 0:512])
            nc.sync.dma_start(xT[:, 0, 512:1024], xt[:, 0, 512:1024])
            wts[1] = wqp.tile([P, DC, P], F32R, tag="wq", name="wt1")
            nc.sync.dma_start(wts[1][:], wqk[:, 1, :, :])
            for dc in range(1, DC):
                nc.sync.dma_start(xT[:, dc, :], xt[:, dc, :])
            nc.sync.dma_start(qs_sb[:], qscale[:])
            nc.vector.memset(ones_row[:], 1.0)
            nc.vector.memset(ones_col[:], 1.0)
            nc.vector.memset(ones_blk[:], 1.0)
            nc.vector.tensor_copy(ones64r[:], ones_row[:, 0:DH])
            nc.vector.tensor_copy(Vt[:, :, :, DH : DH + 1], ones_blk[:])

            # ---- phase A: q,k projections (transposed layout) + raw sum-sq ----
            for ct in range(8):
                if ct in wts:
                    wt = wts.pop(ct)
                else:
                    wt = wqp.tile([P, DC, P], F32R, tag="wq")
                    nc.sync.dma_start(wt[:], wqk[:, ct, :, :])
                ps = psp.tile([P, 2, 512], F32, tag="mm2")
                for st in range(2):
                    for dc in range(DC):
                        nc.tensor.matmul(
                            ps[:, st, :],
                            lhsT=wt[:, dc, :],
                            rhs=xT[:, dc, st * 512 : (st + 1) * 512],
                            start=(dc == 0),
                            stop=(dc == DC - 1),
                        )
                scr = scrp.tile([P, 2, 512], F32, tag="scr")
                nc.scalar.activation(
                    scr[:], ps[:], AF.Square, accum_out=sq_acc[:, ct : ct + 1]
                )
                flat = ps[:].rearrange("p a b -> p (a b)")
                if ct < 4:
                    nc.vector.tensor_scalar(
                        QT[:, ct, :], flat, qs_sb[:, ct : ct + 1], None, ALU.mult
                    )
                else:
                    nc.vector.tensor_copy(KT[:, ct - 4, :], flat)

            # prefetch the sqrt ACT table set while PE is still busy
            nc.scalar.activation(dsq[:], ones_row[:, 0:2], AF.Sqrt)

            # ---- global RMS part 1: reduce + AllReduce + scalar chain ----
            nc.vector.reduce_sum(qk2[:, 0:1], sq_acc[:, 0:4], axis=AX.X)
            nc.vector.reduce_sum(qk2[:, 1:2], sq_acc[:, 4:8], axis=AX.X)
            g_ps = pav.tile([P, 512], F32, tag="av", name="g_ps")
            nc.tensor.matmul(
                g_ps[0:2, 0:1], lhsT=qk2[:], rhs=ones_col[:], start=True, stop=True
            )
            nc.vector.tensor_copy(g_sb[:], g_ps[0:2, 0:1])
            cc_in = dramp.tile([2, 1], F32, tag="ccin")
            cc_out = dramp.tile([2, 1], F32, tag="ccout", addr_space="Shared")
            nc.sync.dma_start(cc_in[:], g_sb[:])
            nc.gpsimd.collective_compute(
                "AllReduce",
                ALU.add,
                replica_groups=REPLICAS,
                ins=[cc_in[:]],
                outs=[cc_out[:]],
            )
            nc.sync.dma_start(gsum[:], cc_out[:].rearrange("a b -> b a"))
            # mean, sqrt (+1 Newton step), +eps, product, reciprocal
            nc.vector.tensor_scalar_mul(sc_a[:], gsum[:], 1.0 / COUNT)  # m
            nc.scalar.activation(sc_b[:], sc_a[:], AF.Sqrt)             # r0
            nc.vector.reciprocal(sc_c[:], sc_b[:])                      # 1/r
            nc.vector.tensor_mul(sc_c[:], sc_a[:], sc_c[:])             # m/r
            nc.vector.tensor_add(sc_b[:], sc_b[:], sc_c[:])             # r + m/r
            nc.vector.tensor_scalar(sc_b[:], sc_b[:], 0.5, EPS, ALU.mult, ALU.add)
            nc.scalar.activation(dsq[:], ones_row[:, 0:2], AF.Exp)
            nc.vector.tensor_mul(pm[:], sc_b[:, 0:1], sc_b[:, 1:2])
            nc.vector.reciprocal(cinv[:], pm[:])

            # ---- phase A2: v projection (covers the collective latency) ----
            Wv_sb = ep.tile([P, DC, NHL * DH], F32R, tag="E", name="Wv_sb")
            nc.sync.dma_start(Wv_sb[:], wv[:])
            for u in range(4):
                ps = psp.tile([P, 2, 512], F32, tag="mm2")
                for j in range(2):
                    sm = 2 * u + j
                    for dc in range(DC):
                        nc.tensor.matmul(
                            ps[:, j, :],
                            lhsT=xT[:, dc, sm * P : (sm + 1) * P],
                            rhs=Wv_sb[:, dc, :],
                            start=(dc == 0),
                            stop=(dc == DC - 1),
                        )
                nc.vector.tensor_copy(
                    Vt[:, 2 * u : 2 * u + 2, :, 0:DH],
                    ps[:].rearrange("p a (h d) -> p a h d", h=NHL),
                )

            # ---- global RMS part 2: broadcast scalar to 128 partitions ----
            nc.gpsimd.partition_broadcast(c_bc[:], cinv[:])
            nc.sync.dma_start(Wo_sb[:], wo[:])

            # ---- PE warm-keepers: cover the AllReduce+chain stall so the
            # HAM clock gate stays at 8/8 entering the attention phase ----
            warm_ps = pav.tile([P, 512], F32, tag="av", name="warm_ps")
            for _ in range(60):
                nc.tensor.matmul(
                    warm_ps[:],
                    lhsT=KT[:, 0, 0:P],
                    rhs=KT[:, 0, 0:512],
                    start=True,
                    stop=True,
                )

            # ---- phase C: attention, head pairs packed on PE row groups ----
            # heads (2g, 2g+1) live at partitions 0:64 / 64:128 of chunk g;
            # alternating their S^T matmuls engages concurrent 64-row tiles.
            for g in (2, 3, 0, 1):
                h0 = 2 * g
                for t in range(2):
                    tsl = slice(t * 512, (t + 1) * 512)
                    E_pair = [
                        ep.tile([P, 8, 512], F32R, tag="E", name=f"E_{g}_{t}_{i}")
                        for i in range(2)
                    ]
                    ps_pair = {}
                    for j in range(4):
                        for i in range(2):
                            ps_pair[(j, i)] = psp.tile(
                                [P, 2, 512], F32, tag="mm2", name=f"s_{j}_{i}"
                            )
                        for jj in range(2):
                            skt = 2 * j + jj
                            for i in range(2):
                                hp = i * DH
                                nc.tensor.matmul(
                                    ps_pair[(j, i)][:, jj, :],
                                    lhsT=KT[hp : hp + DH, g, skt * P : (skt + 1) * P],
                                    rhs=QT[hp : hp + DH, g, tsl],
                                    start=True,
                                    stop=True,
                                )
                        for i in range(2):
                            nc.scalar.activation(
                                E_pair[i][:, 2 * j : 2 * j + 2, :],
                                ps_pair[(j, i)][:],
                                AF.Exp,
                                scale=c_bc[:, 0:1],
                            )
                    for i in range(2):
                        h = h0 + i
                        E_t = E_pair[i]
                        ps_av = pav.tile([P, 512], F32, tag="av", name=f"av_{i}")
                        for skc in range(8):
                            nc.tensor.matmul(
                                ps_av[0 : DH + 1, :],
                                lhsT=Vt[:, skc, h, :],
                                rhs=E_t[:, skc, :],
                                start=(skc == 0),
                                stop=(skc == 7),
                            )
                        rs0 = smallp.tile([1, 512], F32, tag="rs0", name=f"rs0_{i}")
                        nc.vector.tensor_copy(rs0[:], ps_av[DH : DH + 1, :])
                        rs_r = smallp.tile([1, 512], F32, tag="rs", name=f"rs_{i}")
                        nc.vector.reciprocal_approx_fast(rs_r[:], rs0[:])
                        bc_sb = smallp.tile([DH, 512], F32, tag="bcs", name=f"bc_{i}")
                        nc.gpsimd.partition_broadcast(bc_sb[:], rs_r[:])
                        if h < 4:
                            nc.vector.tensor_mul(
                                zT[0:DH, h, tsl], ps_av[0:DH, :], bc_sb[:]
                            )
                        else:
                            ztmp = smallp.tile(
                                [DH, 512], F32R, tag="ztmp", name=f"zt_{i}"
                            )
                            nc.vector.tensor_mul(ztmp[:], ps_av[0:DH, :], bc_sb[:])
                            nc.sync.dma_start(zT[DH:P, h - 4, tsl], ztmp[:])

            # ---- phase D: output projection (partial over local heads) ----
            for sm in range(8):
                ps_o = psp.tile([P, 2, 512], F32, tag="mm2")
                for nt in range(2):
                    for hdc in range(4):
                        nc.tensor.matmul(
                            ps_o[:, nt, :],
                            lhsT=zT[:, hdc, sm * P : (sm + 1) * P],
                            rhs=Wo_sb[:, hdc, nt * 512 : (nt + 1) * 512],
                            start=(hdc == 0),
                            stop=(hdc == 3),
                        )
                ob = scrp.tile([P, 2, 512], F32, tag="scr")
                nc.any.tensor_copy(ob[:], ps_o[:])
                nc.sync.dma_start(
                    zpart[sm * P : (sm + 1) * P, :],
                    ob[:].rearrange("p a b -> p (a b)"),
                )

    nc.compile()
    return nc


def _get_nc():
    if "nc" not in _CACHE:
        _CACHE["nc"] = _build()
    return _CACHE["nc"]


def _prep_core_inputs(x, Wqkv, Wo, scale_q, scale_k):
    """Host-side shard + layout prep. Returns list of 8 in_maps."""
    x = np.asarray(x, dtype=np.float32)
    Wqkv = np.asarray(Wqkv, dtype=np.float32)
    Wo = np.asarray(Wo, dtype=np.float32)
    scale_q = np.asarray(scale_q, dtype=np.float32)
    scale_k = np.asarray(scale_k, dtype=np.float32)

    # combined per-d_head scale folded into Q (applied after raw sum-sq)
    qs_vec = np.tile(scale_q * scale_k, NHL)          # [512]
    qs_dev = np.ascontiguousarray(qs_vec.reshape(4, P).T)  # [128,4]

    xt_all = []
    for b in range(4):
        xTb = _rne11(x[b].T)                           # [d, s]
        xt_all.append(
            np.ascontiguousarray(xTb.reshape(DC, P, S).transpose(1, 0, 2))
        )  # [128, 8, 1024]

    in_maps = []
    for c in range(8):
        b = c // 2
        hh = (c % 2) * NHL
        cols = slice(hh * DH, (hh + NHL) * DH)
        wq_c = Wqkv[:, 0 * D:1 * D][:, cols]           # [1024, 512]
        wk_c = Wqkv[:, 1 * D:2 * D][:, cols]
        wv_c = Wqkv[:, 2 * D:3 * D][:, cols]
        wqk_c = _rne11(np.concatenate([wq_c, wk_c], axis=1))  # [1024, 1024]
        # [p, ct, dc, n]: per-ct slices are contiguous 4KB-per-partition DMAs
        wqk_dev = np.ascontiguousarray(
            wqk_c.reshape(DC, P, 8, P).transpose(1, 2, 0, 3)
        )
        wv_dev = np.ascontiguousarray(
            _rne11(wv_c).reshape(DC, P, NHL * DH).transpose(1, 0, 2)
        )
        # Wo rows for local heads, arranged [128, 4, 1024]:
        # chunk k low half = head k, high half = head 4+k
        wo_loc = _rne11(Wo[(hh * DH):(hh + NHL) * DH, :])   # [512, 1024]
        wo_dev = np.empty((P, 4, D), dtype=np.float32)
        for k in range(4):
            wo_dev[0:DH, k, :] = wo_loc[k * DH:(k + 1) * DH, :]
            wo_dev[DH:P, k, :] = wo_loc[(4 + k) * DH:(5 + k) * DH, :]
        in_maps.append(
            {
                "xt": xt_all[b],
                "wqk": wqk_dev,
                "wv": wv_dev,
                "wo": np.ascontiguousarray(wo_dev),
                "qscale": qs_dev,
            }
        )
    return in_maps


def run(x, Wqkv, Wo, scale_q, scale_k, trace=False):
    nc = _get_nc()
    in_maps = _prep_core_inputs(x, Wqkv, Wo, scale_q, scale_k)
    res = run_bass_kernel_spmd(
        nc, in_maps[:N_CORES], core_ids=list(range(N_CORES)), trace=trace
    )
    out = np.empty((4, S, D), dtype=np.float32)
    for b in range(4):
        if N_CORES == 8:
            out[b] = res.results[2 * b]["zpart"] + res.results[2 * b + 1]["zpart"]
    return out, res


def kernel(x, Wqkv, Wo, scale_q, scale_k):
    out, _ = run(x, Wqkv, Wo, scale_q, scale_k, trace=False)
    return out


# revision 29
# speedup vs baseline: 1.0526x; 1.0424x over previous
"""TRN2 Bass kernel for nn_Attention_188978561266.

Reference computation (b=4, s=1024, d=1024, 16 heads x 64):
    qkv = x @ Wqkv ; split q,k,v
    q = q / (sqrt(mean(q^2 over ALL elements)) + eps) * scale_q   (global scalar RMS)
    k = k / (sqrt(mean(k^2 over ALL elements)) + eps) * scale_k
    attn = softmax(q @ k^T)  (no 1/sqrt(d_head), no mask)
    out = (attn @ v) @ Wo

Sharding: 8 cores = (batch b in 0..3) x (head-half in 0..1). Each core computes
qkv for its batch restricted to its 8 heads (tensor-parallel QKV columns),
full attention for those heads, and a partial output projection. Host sums
the two partial outputs per batch. The global RMS needs a cross-core
AllReduce of two scalars (sum q^2, sum k^2).

Projection/logit matmuls run in float32r (full PE rate, 11-bit mantissa RNE —
bit-exact vs host pre-rounding), accumulating in fp32 PSUM. The attention
weights E=exp(S) and V default to bf16 (same exponent range as fp32, so no
overflow; set KE_BF16=0 for f32r E/V at ~8e-4 rel err vs ~1.9e-3). The
per-d_head scale_q*scale_k vector folds into Q at the psum->SBUF copy; the
runtime 1/((rms_q+eps)(rms_k+eps)) scalar folds into the softmax exp's scale
operand, so the PE never waits on the collective for the S = K^T Q matmuls.
A dummy AllReduce at kernel start pre-warms the collective firmware and
absorbs cross-core start skew; warm-keeper matmuls cover the real AllReduce
so the PE clock gate stays hot into the attention phase.
"""

import sys

sys.path.insert(0, "/opt/trn_rl_repo")

import numpy as np

import concourse.bacc as bacc
import concourse.mybir as mybir
from concourse import library_config, tile
from concourse.bass_utils import run_bass_kernel_spmd

F32 = mybir.dt.float32
F32R = mybir.dt.float32r
BF16 = mybir.dt.bfloat16
E_DT = BF16 if _os.environ.get("KE_BF16", "1") == "1" else F32R
AF = mybir.ActivationFunctionType
ALU = mybir.AluOpType
AX = mybir.AxisListType

P = 128
D = 1024
S = 1024
N_HEAD = 16
DH = 64
NHL = 8          # heads per core
DC = 8           # d contraction chunks of 128
EPS = 1e-6
COUNT = 4 * 1024 * 1024   # elements of the full q (or k) tensor
import os as _os
N_CORES = int(_os.environ.get("KN_CORES", "8"))
REPLICAS = [list(range(N_CORES))]

_CACHE = {}


def _rne11(x: np.ndarray) -> np.ndarray:
    """Round float32 to 11 explicit mantissa bits (matches HW float32r)."""
    u = np.ascontiguousarray(x, dtype=np.float32).view(np.uint32).astype(np.uint64)
    shift = 12
    bias = ((u >> shift) & 1) + ((1 << (shift - 1)) - 1)
    return (((u + bias) >> shift) << shift).astype(np.uint32).view(np.float32)


def _build():
    nc = bacc.Bacc("TRN2", target_bir_lowering=False, debug=False, num_devices=N_CORES)

    xt = nc.dram_tensor("xt", [P, DC, S], F32R, kind="ExternalInput")
    wqk = nc.dram_tensor("wqk", [P, 8, DC, P], F32R, kind="ExternalInput")
    wv = nc.dram_tensor("wv", [P, DC, NHL * DH], F32R, kind="ExternalInput")
    wo = nc.dram_tensor("wo", [P, 4, D], F32R, kind="ExternalInput")
    qscale = nc.dram_tensor("qscale", [P, 4], F32, kind="ExternalInput")
    zpart = nc.dram_tensor("zpart", [S, D], F32, kind="ExternalOutput")

    with tile.TileContext(nc) as tc:
        with (
            tc.tile_pool(name="big", bufs=1) as big,
            tc.tile_pool(name="wq", bufs=3) as wqp,
            tc.tile_pool(name="ep", bufs=4) as ep,
            tc.tile_pool(name="scr", bufs=2) as scrp,
            tc.tile_pool(name="small", bufs=2) as smallp,
            tc.tile_pool(name="stats", bufs=1) as stp,
            tc.tile_pool(name="ps", bufs=3, space="PSUM") as psp,
            tc.tile_pool(name="pav", bufs=2, space="PSUM") as pav,
            tc.tile_pool(name="dram", bufs=1, space="DRAM") as dramp,
        ):
            # ---- persistent SBUF tensors ----
            xT = big.tile([P, DC, S], F32R, tag="xT")
            QT = big.tile([P, 4, S], F32R, tag="QT")
            KT = big.tile([P, 4, S], F32R, tag="KT")
            Vt = big.tile([P, 8, NHL, DH + 1], E_DT, tag="Vt")
            zT = big.tile([P, 4, S], F32R, tag="zT")

            Wo_sb = big.tile([P, 4, D], F32R, tag="Wo")

            qs_sb = stp.tile([P, 4], F32, tag="qs")
            sq_acc = stp.tile([P, 8], F32, tag="sqacc")
            qk2 = stp.tile([P, 2], F32, tag="qk2")
            g_sb = stp.tile([2, 1], F32, tag="gsb")
            gsum = stp.tile([1, 2], F32, tag="gsum")
            sc_a = stp.tile([1, 2], F32, tag="sca")
            sc_b = stp.tile([1, 2], F32, tag="scb")
            sc_c = stp.tile([1, 2], F32, tag="scc")
            pm = stp.tile([1, 1], F32, tag="pm")
            cinv = stp.tile([1, 1], F32, tag="cinv")
            c_bc = stp.tile([P, 1], F32, tag="cbc")

            dsq = stp.tile([1, 2], F32, tag="dsq")
            ones_row = stp.tile([1, P], F32, tag="ones_row")
            ones_col = stp.tile([P, 1], F32, tag="ones_col")
            ones_blk = stp.tile([P, 8, NHL, 1], F32, tag="ones_blk")

            nc.gpsimd.load_library(library_config.attn)
            cc_warm_in = dramp.tile([2, 1], F32, tag="ccwi")
            cc_warm_out = dramp.tile([2, 1], F32, tag="ccwo", addr_space="Shared")
            nc.gpsimd.collective_compute(
                "AllReduce",
                ALU.add,
                replica_groups=REPLICAS,
                ins=[cc_warm_in[:]],
                outs=[cc_warm_out[:]],
            )
            # ---- input DMAs + consts (first weight tiles before bulk x) ----
            wts = {}
            wts[0] = wqp.tile([P, DC, P], F32R, tag="wq", name="wt0")
            nc.sync.dma_start(wts[0][:], wqk[:, 0, :, :])
            nc.sync.dma_start(xT[:, 0, 0:512], xt[:, 0, 0:512])
            nc.sync.dma_start(xT[:, 0, 512:1024], xt[:, 0, 512:1024])
            wts[1] = wqp.tile([P, DC, P], F32R, tag="wq", name="wt1")
            nc.sync.dma_start(wts[1][:], wqk[:, 1, :, :])
            for dc in range(1, DC):
                nc.sync.dma_start(xT[:, dc, 0:512], xt[:, dc, 0:512])
                nc.sync.dma_start(xT[:, dc, 512:1024], xt[:, dc, 512:1024])
            nc.sync.dma_start(qs_sb[:], qscale[:])
            nc.vector.memset(ones_row[:], 1.0)
            nc.vector.memset(ones_col[:], 1.0)
            nc.vector.memset(ones_blk[:], 1.0)
            nc.vector.tensor_copy(Vt[:, :, :, DH : DH + 1], ones_blk[:])

            # ---- phase A: q,k projections (transposed layout) + raw sum-sq ----
            for ct in range(8):
                if ct in wts:
                    wt = wts.pop(ct)
                else:
                    wt = wqp.tile([P, DC, P], F32R, tag="wq")
                    nc.sync.dma_start(wt[:], wqk[:, ct, :, :])
                ps = psp.tile([P, 2, 512], F32, tag="mm2")
                for st in range(2):
                    for dc in range(DC):
                        nc.tensor.matmul(
                            ps[:, st, :],
                            lhsT=wt[:, dc, :],
                            rhs=xT[:, dc, st * 512 : (st + 1) * 512],
                            start=(dc == 0),
                            stop=(dc == DC - 1),
                        )
                scr = scrp.tile([P, 2, 512], F32, tag="scr")
                nc.scalar.activation(
                    scr[:], ps[:], AF.Square, accum_out=sq_acc[:, ct : ct + 1]
                )
                flat = ps[:].rearrange("p a b -> p (a b)")
                if ct < 4:
                    nc.vector.tensor_scalar(
                        QT[:, ct, :], flat, qs_sb[:, ct : ct + 1], None, ALU.mult
                    )
                else:
                    nc.vector.tensor_copy(KT[:, ct - 4, :], flat)

            # prefetch the sqrt ACT table set while PE is still busy
            nc.scalar.activation(dsq[:], ones_row[:, 0:2], AF.Sqrt)

            # ---- global RMS part 1: reduce + AllReduce + scalar chain ----
            nc.vector.reduce_sum(qk2[:, 0:1], sq_acc[:, 0:4], axis=AX.X)
            nc.vector.reduce_sum(qk2[:, 1:2], sq_acc[:, 4:8], axis=AX.X)
            g_ps = pav.tile([P, 512], F32, tag="av", name="g_ps")
            nc.tensor.matmul(
                g_ps[0:2, 0:1], lhsT=qk2[:], rhs=ones_col[:], start=True, stop=True
            )
            nc.vector.tensor_copy(g_sb[:], g_ps[0:2, 0:1])
            cc_in = dramp.tile([2, 1], F32, tag="ccin")
            cc_out = dramp.tile([2, 1], F32, tag="ccout", addr_space="Shared")
            nc.sync.dma_start(cc_in[:], g_sb[:])
            nc.gpsimd.collective_compute(
                "AllReduce",
                ALU.add,
                replica_groups=REPLICAS,
                ins=[cc_in[:]],
                outs=[cc_out[:]],
            )
            nc.sync.dma_start(gsum[:], cc_out[:].rearrange("a b -> b a"))
            # mean, sqrt (+1 Newton step), +eps, product, reciprocal
            nc.vector.tensor_scalar_mul(sc_a[:], gsum[:], 1.0 / COUNT)  # m
            nc.scalar.activation(sc_b[:], sc_a[:], AF.Sqrt)             # r0
            nc.vector.reciprocal(sc_c[:], sc_b[:])                      # 1/r
            nc.vector.tensor_mul(sc_c[:], sc_a[:], sc_c[:])             # m/r
            nc.vector.tensor_add(sc_b[:], sc_b[:], sc_c[:])             # r + m/r
            nc.vector.tensor_scalar(sc_b[:], sc_b[:], 0.5, EPS, ALU.mult, ALU.add)
            nc.scalar.activation(dsq[:], ones_row[:, 0:2], AF.Exp)
            nc.vector.tensor_mul(pm[:], sc_b[:, 0:1], sc_b[:, 1:2])
            nc.vector.reciprocal(cinv[:], pm[:])

            # ---- phase A2: v projection (covers the collective latency) ----
            Wv_sb = ep.tile([P, DC, NHL * DH], F32R, tag="E", name="Wv_sb")
            nc.sync.dma_start(Wv_sb[:], wv[:])
            for u in range(4):
                ps = psp.tile([P, 2, 512], F32, tag="mm2")
                for j in range(2):
                    sm = 2 * u + j
                    for dc in range(DC):
                        nc.tensor.matmul(
                            ps[:, j, :],
                            lhsT=xT[:, dc, sm * P : (sm + 1) * P],
                            rhs=Wv_sb[:, dc, :],
                            start=(dc == 0),
                            stop=(dc == DC - 1),
                        )
                nc.vector.tensor_copy(
                    Vt[:, 2 * u : 2 * u + 2, :, 0:DH],
                    ps[:].rearrange("p a (h d) -> p a h d", h=NHL),
                )

            # ---- global RMS part 2: broadcast scalar to 128 partitions ----
            nc.gpsimd.partition_broadcast(c_bc[:], cinv[:])
            nc.sync.dma_start(Wo_sb[:], wo[:])

            # ---- PE warm-keepers: cover the AllReduce+chain stall so the
            # HAM clock gate stays at 8/8 entering the attention phase ----
            warm_ps = pav.tile([P, 512], F32, tag="av", name="warm_ps")
            for _ in range(60):
                nc.tensor.matmul(
                    warm_ps[:],
                    lhsT=KT[:, 0, 0:P],
                    rhs=KT[:, 0, 0:512],
                    start=True,
                    stop=True,
                )

            # ---- phase C: attention, head pairs packed on PE row groups ----
            # heads (2g, 2g+1) live at partitions 0:64 / 64:128 of chunk g;
            # alternating their S^T matmuls engages concurrent 64-row tiles.
            for g in (2, 3, 0, 1):
                h0 = 2 * g
                for t in range(2):
                    tsl = slice(t * 512, (t + 1) * 512)
                    E_pair = [
                        ep.tile([P, 8, 512], E_DT, tag="E", name=f"E_{g}_{t}_{i}")
                        for i in range(2)
                    ]
                    ps_pair = {}
                    for j in range(4):
                        for i in range(2):
                            ps_pair[(j, i)] = psp.tile(
                                [P, 2, 512], F32, tag="mm2", name=f"s_{j}_{i}"
                            )
                        for jj in range(2):
                            skt = 2 * j + jj
                            for i in range(2):
                                hp = i * DH
                                nc.tensor.matmul(
                                    ps_pair[(j, i)][:, jj, :],
                                    lhsT=KT[hp : hp + DH, g, skt * P : (skt + 1) * P],
                                    rhs=QT[hp : hp + DH, g, tsl],
                                    start=True,
                                    stop=True,
                                )
                        for i in range(2):
                            nc.scalar.activation(
                                E_pair[i][:, 2 * j : 2 * j + 2, :],
                                ps_pair[(j, i)][:],
                                AF.Exp,
                                scale=c_bc[:, 0:1],
                            )
                    for i in range(2):
                        h = h0 + i
                        E_t = E_pair[i]
                        ps_av = pav.tile([P, 512], F32, tag="av", name=f"av_{i}")
                        for skc in range(8):
                            nc.tensor.matmul(
                                ps_av[0 : DH + 1, :],
                                lhsT=Vt[:, skc, h, :],
                                rhs=E_t[:, skc, :],
                                start=(skc == 0),
                                stop=(skc == 7),
                            )
                        rs0 = smallp.tile([1, 512], F32, tag="rs0", name=f"rs0_{i}")
                        nc.vector.tensor_copy(rs0[:], ps_av[DH : DH + 1, :])
                        rs_r = smallp.tile([1, 512], F32, tag="rs", name=f"rs_{i}")
                        nc.vector.reciprocal_approx_fast(rs_r[:], rs0[:])
                        bc_sb = smallp.tile([DH, 512], F32, tag="bcs", name=f"bc_{i}")
                        nc.gpsimd.partition_broadcast(bc_sb[:], rs_r[:])
                        if h < 4:
                            nc.vector.tensor_mul(
                                zT[0:DH, h, tsl], ps_av[0:DH, :], bc_sb[:]
                            )
                        else:
                            ztmp = smallp.tile(
                                [DH, 512], F32R, tag="ztmp", name=f"zt_{i}"
                            )
                            nc.vector.tensor_mul(ztmp[:], ps_av[0:DH, :], bc_sb[:])
                            nc.sync.dma_start(zT[DH:P, h - 4, tsl], ztmp[:])

            # ---- phase D: output projection (partial over local heads) ----
            for sm in range(8):
                ps_o = psp.tile([P, 2, 512], F32, tag="mm2")
                for nt in range(2):
                    for hdc in range(4):
                        nc.tensor.matmul(
                            ps_o[:, nt, :],
                            lhsT=zT[:, hdc, sm * P : (sm + 1) * P],
                            rhs=Wo_sb[:, hdc, nt * 512 : (nt + 1) * 512],
                            start=(hdc == 0),
                            stop=(hdc == 3),
                        )
                ob = scrp.tile([P, 2, 512], F32, tag="scr")
                nc.vector.tensor_copy(ob[:], ps_o[:])
                nc.sync.dma_start(
                    zpart[sm * P : (sm + 1) * P, :],
                    ob[:].rearrange("p a b -> p (a b)"),
                )

    nc.compile()
    return nc


def _get_nc():
    if "nc" not in _CACHE:
        _CACHE["nc"] = _build()
    return _CACHE["nc"]


def _prep_core_inputs(x, Wqkv, Wo, scale_q, scale_k):
    """Host-side shard + layout prep. Returns list of 8 in_maps."""
    x = np.asarray(x, dtype=np.float32)
    Wqkv = np.asarray(Wqkv, dtype=np.float32)
    Wo = np.asarray(Wo, dtype=np.float32)
    scale_q = np.asarray(scale_q, dtype=np.float32)
    scale_k = np.asarray(scale_k, dtype=np.float32)

    # combined per-d_head scale folded into Q (applied after raw sum-sq)
    qs_vec = np.tile(scale_q * scale_k, NHL)          # [512]
    qs_dev = np.ascontiguousarray(qs_vec.reshape(4, P).T)  # [128,4]

    xt_all = []
    for b in range(4):
        xTb = _rne11(x[b].T)                           # [d, s]
        xt_all.append(
            np.ascontiguousarray(xTb.reshape(DC, P, S).transpose(1, 0, 2))
        )  # [128, 8, 1024]

    in_maps = []
    for c in range(8):
        b = c // 2
        hh = (c % 2) * NHL
        cols = slice(hh * DH, (hh + NHL) * DH)
        wq_c = Wqkv[:, 0 * D:1 * D][:, cols]           # [1024, 512]
        wk_c = Wqkv[:, 1 * D:2 * D][:, cols]
        wv_c = Wqkv[:, 2 * D:3 * D][:, cols]
        wqk_c = _rne11(np.concatenate([wq_c, wk_c], axis=1))  # [1024, 1024]
        # [p, ct, dc, n]: per-ct slices are contiguous 4KB-per-partition DMAs
        wqk_dev = np.ascontiguousarray(
            wqk_c.reshape(DC, P, 8, P).transpose(1, 2, 0, 3)
        )
        wv_dev = np.ascontiguousarray(
            _rne11(wv_c).reshape(DC, P, NHL * DH).transpose(1, 0, 2)
        )
        # Wo rows for local heads, arranged [128, 4, 1024]:
        # chunk k low half = head k, high half = head 4+k
        wo_loc = _rne11(Wo[(hh * DH):(hh + NHL) * DH, :])   # [512, 1024]
        wo_dev = np.empty((P, 4, D), dtype=np.float32)
        for k in range(4):
            wo_dev[0:DH, k, :] = wo_loc[k * DH:(k + 1) * DH, :]
            wo_dev[DH:P, k, :] = wo_loc[(4 + k) * DH:(5 + k) * DH, :]
        in_maps.append(
            {
                "xt": xt_all[b],
                "wqk": wqk_dev,
                "wv": wv_dev,
                "wo": np.ascontiguousarray(wo_dev),
                "qscale": qs_dev,
            }
        )
    return in_maps


def run(x, Wqkv, Wo, scale_q, scale_k, trace=False):
    nc = _get_nc()
    in_maps = _prep_core_inputs(x, Wqkv, Wo, scale_q, scale_k)
    res = run_bass_kernel_spmd(
        nc, in_maps[:N_CORES], core_ids=list(range(N_CORES)), trace=trace
    )
    out = np.empty((4, S, D), dtype=np.float32)
    for b in range(4):
        if N_CORES == 8:
            out[b] = res.results[2 * b]["zpart"] + res.results[2 * b + 1]["zpart"]
    return out, res


def kernel(x, Wqkv, Wo, scale_q, scale_k):
    out, _ = run(x, Wqkv, Wo, scale_q, scale_k, trace=False)
    return out


# revision 32
# speedup vs baseline: 1.1142x; 1.0586x over previous
"""TRN2 Bass kernel for nn_Attention_188978561266.

Reference computation (b=4, s=1024, d=1024, 16 heads x 64):
    qkv = x @ Wqkv ; split q,k,v
    q = q / (sqrt(mean(q^2 over ALL elements)) + eps) * scale_q   (global scalar RMS)
    k = k / (sqrt(mean(k^2 over ALL elements)) + eps) * scale_k
    attn = softmax(q @ k^T)  (no 1/sqrt(d_head), no mask)
    out = (attn @ v) @ Wo

Sharding: 8 cores = (batch b in 0..3) x (head-half in 0..1). Each core computes
qkv for its batch restricted to its 8 heads (tensor-parallel QKV columns),
full attention for those heads, and a partial output projection. Host sums
the two partial outputs per batch. The global RMS needs a cross-core
AllReduce of two scalars (sum q^2, sum k^2).

Projection/logit matmuls run in float32r (full PE rate, 11-bit mantissa RNE —
bit-exact vs host pre-rounding), accumulating in fp32 PSUM. The attention
weights E=exp(S) and V default to bf16 (same exponent range as fp32, so no
overflow; set KE_BF16=0 for f32r E/V at ~8e-4 rel err vs ~1.9e-3). The
per-d_head scale_q*scale_k vector folds into Q at the psum->SBUF copy; the
runtime 1/((rms_q+eps)(rms_k+eps)) scalar folds into the softmax exp's scale
operand, so the PE never waits on the collective for the S = K^T Q matmuls.
A dummy AllReduce at kernel start pre-warms the collective firmware and
absorbs cross-core start skew; warm-keeper matmuls cover the real AllReduce
so the PE clock gate stays hot into the attention phase.
"""

import sys

sys.path.insert(0, "/opt/trn_rl_repo")

import numpy as np

import concourse.bacc as bacc
import concourse.mybir as mybir
from concourse import library_config, tile
from concourse.bass_utils import run_bass_kernel_spmd

F32 = mybir.dt.float32
F32R = mybir.dt.float32r
BF16 = mybir.dt.bfloat16
E_DT = BF16 if _os.environ.get("KE_BF16", "1") == "1" else F32R
AF = mybir.ActivationFunctionType
ALU = mybir.AluOpType
AX = mybir.AxisListType

P = 128
D = 1024
S = 1024
N_HEAD = 16
DH = 64
NHL = 8          # heads per core
DC = 8           # d contraction chunks of 128
EPS = 1e-6
COUNT = 4 * 1024 * 1024   # elements of the full q (or k) tensor
import os as _os
N_CORES = int(_os.environ.get("KN_CORES", "8"))
REPLICAS = [list(range(N_CORES))]

_CACHE = {}


def _rne11(x: np.ndarray) -> np.ndarray:
    """Round float32 to 11 explicit mantissa bits (matches HW float32r)."""
    u = np.ascontiguousarray(x, dtype=np.float32).view(np.uint32).astype(np.uint64)
    shift = 12
    bias = ((u >> shift) & 1) + ((1 << (shift - 1)) - 1)
    return (((u + bias) >> shift) << shift).astype(np.uint32).view(np.float32)


def _build():
    nc = bacc.Bacc("TRN2", target_bir_lowering=False, debug=False, num_devices=N_CORES)

    xt = nc.dram_tensor("xt", [P, DC, S], F32R, kind="ExternalInput")
    wqk = nc.dram_tensor("wqk", [P, 8, DC, P], F32R, kind="ExternalInput")
    wv = nc.dram_tensor("wv", [P, DC, NHL * DH], F32R, kind="ExternalInput")
    wo = nc.dram_tensor("wo", [P, 4, D], F32R, kind="ExternalInput")
    qscale = nc.dram_tensor("qscale", [P, 4], F32, kind="ExternalInput")
    zpart = nc.dram_tensor("zpart", [S, D], F32, kind="ExternalOutput")

    with tile.TileContext(nc) as tc:
        with (
            tc.tile_pool(name="big", bufs=1) as big,
            tc.tile_pool(name="wq", bufs=3) as wqp,
            tc.tile_pool(name="ep", bufs=4) as ep,
            tc.tile_pool(name="scr", bufs=2) as scrp,
            tc.tile_pool(name="small", bufs=2) as smallp,
            tc.tile_pool(name="stats", bufs=1) as stp,
            tc.tile_pool(name="ps", bufs=3, space="PSUM") as psp,
            tc.tile_pool(name="pav", bufs=2, space="PSUM") as pav,
            tc.tile_pool(name="dram", bufs=1, space="DRAM") as dramp,
        ):
            # ---- persistent SBUF tensors ----
            xT = big.tile([P, DC, S], F32R, tag="xT")
            QT = big.tile([P, 4, S], F32R, tag="QT")
            KT = big.tile([P, 4, S], F32R, tag="KT")
            Vt = big.tile([P, 8, NHL, DH + 1], E_DT, tag="Vt")
            zT = big.tile([P, 4, S], F32R, tag="zT")

            Wo_sb = big.tile([P, 4, D], F32R, tag="Wo")

            qs_sb = stp.tile([P, 4], F32, tag="qs")
            sq_acc = stp.tile([P, 8], F32, tag="sqacc")
            qk2 = stp.tile([P, 2], F32, tag="qk2")
            g_sb = stp.tile([2, 1], F32, tag="gsb")
            gsum = stp.tile([1, 2], F32, tag="gsum")
            sc_a = stp.tile([1, 2], F32, tag="sca")
            sc_b = stp.tile([1, 2], F32, tag="scb")
            sc_c = stp.tile([1, 2], F32, tag="scc")
            pm = stp.tile([1, 1], F32, tag="pm")
            cinv = stp.tile([1, 1], F32, tag="cinv")
            c_bc = stp.tile([P, 1], F32, tag="cbc")

            dsq = stp.tile([1, 2], F32, tag="dsq")
            ones_row = stp.tile([1, P], F32, tag="ones_row")
            ones_col = stp.tile([P, 1], F32, tag="ones_col")
            ones_blk = stp.tile([P, 8, NHL, 1], F32, tag="ones_blk")

            nc.gpsimd.load_library(library_config.attn)
            cc_warm_in = dramp.tile([2, 1], F32, tag="ccwi")
            cc_warm_out = dramp.tile([2, 1], F32, tag="ccwo", addr_space="Shared")
            nc.gpsimd.collective_compute(
                "AllReduce",
                ALU.add,
                replica_groups=REPLICAS,
                ins=[cc_warm_in[:]],
                outs=[cc_warm_out[:]],
            )
            # ---- input DMAs + consts (first weight tiles before bulk x) ----
            wts = {}
            wts[0] = wqp.tile([P, DC, P], F32R, tag="wq", name="wt0")
            nc.sync.dma_start(wts[0][:], wqk[:, 0, :, :])
            nc.sync.dma_start(xT[:, 0, 0:512], xt[:, 0, 0:512])
            nc.sync.dma_start(xT[:, 0, 512:1024], xt[:, 0, 512:1024])
            wts[1] = wqp.tile([P, DC, P], F32R, tag="wq", name="wt1")
            nc.sync.dma_start(wts[1][:], wqk[:, 1, :, :])
            for dc in range(1, DC):
                nc.sync.dma_start(xT[:, dc, 0:512], xt[:, dc, 0:512])
                nc.sync.dma_start(xT[:, dc, 512:1024], xt[:, dc, 512:1024])
            nc.sync.dma_start(qs_sb[:], qscale[:])
            nc.vector.memset(ones_row[:], 1.0)
            nc.vector.memset(ones_col[:], 1.0)
            nc.vector.memset(ones_blk[:], 1.0)
            nc.vector.tensor_copy(Vt[:, :, :, DH : DH + 1], ones_blk[:])

            # ---- phase A: q,k projections (transposed layout) + raw sum-sq ----
            for ct in range(8):
                if ct in wts:
                    wt = wts.pop(ct)
                else:
                    wt = wqp.tile([P, DC, P], F32R, tag="wq")
                    nc.sync.dma_start(wt[:], wqk[:, ct, :, :])
                ps = psp.tile([P, 2, 512], F32, tag="mm2")
                for st in range(2):
                    for dc in range(DC):
                        nc.tensor.matmul(
                            ps[:, st, :],
                            lhsT=wt[:, dc, :],
                            rhs=xT[:, dc, st * 512 : (st + 1) * 512],
                            start=(dc == 0),
                            stop=(dc == DC - 1),
                        )
                scr = scrp.tile([P, 2, 512], F32, tag="scr")
                nc.scalar.activation(
                    scr[:], ps[:], AF.Square, accum_out=sq_acc[:, ct : ct + 1]
                )
                flat = ps[:].rearrange("p a b -> p (a b)")
                if ct < 4:
                    nc.vector.tensor_scalar(
                        QT[:, ct, :], flat, qs_sb[:, ct : ct + 1], None, ALU.mult
                    )
                else:
                    nc.vector.tensor_copy(KT[:, ct - 4, :], flat)

            # prefetch the sqrt ACT table set while PE is still busy
            nc.scalar.activation(dsq[:], ones_row[:, 0:2], AF.Sqrt)

            # ---- global RMS part 1: reduce + AllReduce + scalar chain ----
            nc.vector.reduce_sum(qk2[:, 0:1], sq_acc[:, 0:4], axis=AX.X)
            nc.vector.reduce_sum(qk2[:, 1:2], sq_acc[:, 4:8], axis=AX.X)
            g_ps = pav.tile([P, 512], F32, tag="av", name="g_ps")
            nc.tensor.matmul(
                g_ps[0:2, 0:1], lhsT=qk2[:], rhs=ones_col[:], start=True, stop=True
            )
            nc.vector.tensor_copy(g_sb[:], g_ps[0:2, 0:1])
            cc_in = dramp.tile([2, 1], F32, tag="ccin")
            cc_out = dramp.tile([2, 1], F32, tag="ccout", addr_space="Shared")
            nc.sync.dma_start(cc_in[:], g_sb[:])
            nc.gpsimd.collective_compute(
                "AllReduce",
                ALU.add,
                replica_groups=REPLICAS,
                ins=[cc_in[:]],
                outs=[cc_out[:]],
            )
            nc.sync.dma_start(gsum[:], cc_out[:].rearrange("a b -> b a"))
            # mean, sqrt (+1 Newton step), +eps, product, reciprocal
            nc.vector.tensor_scalar_mul(sc_a[:], gsum[:], 1.0 / COUNT)  # m
            nc.scalar.activation(sc_b[:], sc_a[:], AF.Sqrt)             # r0
            nc.vector.reciprocal(sc_c[:], sc_b[:])                      # 1/r
            nc.vector.tensor_mul(sc_c[:], sc_a[:], sc_c[:])             # m/r
            nc.vector.tensor_add(sc_b[:], sc_b[:], sc_c[:])             # r + m/r
            nc.vector.tensor_scalar(sc_b[:], sc_b[:], 0.5, EPS, ALU.mult, ALU.add)
            nc.scalar.activation(dsq[:], ones_row[:, 0:2], AF.Exp)
            nc.vector.tensor_mul(pm[:], sc_b[:, 0:1], sc_b[:, 1:2])
            nc.vector.reciprocal(cinv[:], pm[:])

            # ---- phase A2: v projection (covers the collective latency) ----
            Wv_sb = ep.tile([P, DC, NHL * DH], F32R, tag="E", name="Wv_sb")
            nc.sync.dma_start(Wv_sb[:], wv[:])
            for u in range(4):
                ps = psp.tile([P, 2, 512], F32, tag="mm2")
                for j in range(2):
                    sm = 2 * u + j
                    for dc in range(DC):
                        nc.tensor.matmul(
                            ps[:, j, :],
                            lhsT=xT[:, dc, sm * P : (sm + 1) * P],
                            rhs=Wv_sb[:, dc, :],
                            start=(dc == 0),
                            stop=(dc == DC - 1),
                        )
                nc.vector.tensor_copy(
                    Vt[:, 2 * u : 2 * u + 2, :, 0:DH],
                    ps[:].rearrange("p a (h d) -> p a h d", h=NHL),
                )

            # ---- global RMS part 2: broadcast scalar to 128 partitions ----
            nc.gpsimd.partition_broadcast(c_bc[:], cinv[:])
            nc.sync.dma_start(Wo_sb[:], wo[:])

            # ---- PE warm-keepers: cover the AllReduce+chain stall so the
            # HAM clock gate stays at 8/8 entering the attention phase ----
            warm_ps = pav.tile([P, 512], F32, tag="av", name="warm_ps")
            for _ in range(60):
                nc.tensor.matmul(
                    warm_ps[:],
                    lhsT=KT[:, 0, 0:P],
                    rhs=KT[:, 0, 0:512],
                    start=True,
                    stop=True,
                )

            # ---- phase C: attention, head pairs packed on PE row groups ----
            # heads (2g, 2g+1) live at partitions 0:64 / 64:128 of chunk g;
            # alternating their S^T matmuls engages concurrent 64-row tiles.
            for g in (2, 3, 0, 1):
                h0 = 2 * g
                for t in range(2):
                    tsl = slice(t * 512, (t + 1) * 512)
                    E_pair = [
                        ep.tile([P, 8, 512], E_DT, tag="E", name=f"E_{g}_{t}_{i}")
                        for i in range(2)
                    ]
                    ps_pair = {}
                    for j in range(4):
                        for i in range(2):
                            ps_pair[(j, i)] = psp.tile(
                                [P, 2, 512], F32, tag="mm2", name=f"s_{j}_{i}"
                            )
                        for jj in range(2):
                            skt = 2 * j + jj
                            for i in range(2):
                                hp = i * DH
                                nc.tensor.matmul(
                                    ps_pair[(j, i)][:, jj, :],
                                    lhsT=KT[hp : hp + DH, g, skt * P : (skt + 1) * P],
                                    rhs=QT[hp : hp + DH, g, tsl],
                                    start=True,
                                    stop=True,
                                )
                        for i in range(2):
                            nc.scalar.activation(
                                E_pair[i][:, 2 * j : 2 * j + 2, :],
                                ps_pair[(j, i)][:],
                                AF.Exp,
                                scale=c_bc[:, 0:1],
                            )
                    for i in range(2):
                        h = h0 + i
                        E_t = E_pair[i]
                        ps_av = pav.tile([P, 512], F32, tag="av", name=f"av_{i}")
                        for skc in range(8):
                            nc.tensor.matmul(
                                ps_av[0 : DH + 1, :],
                                lhsT=Vt[:, skc, h, :],
                                rhs=E_t[:, skc, :],
                                start=(skc == 0),
                                stop=(skc == 7),
                            )
                        rs0 = smallp.tile([1, 512], F32, tag="rs0", name=f"rs0_{i}")
                        nc.vector.tensor_copy(rs0[:], ps_av[DH : DH + 1, :])
                        rs_r = smallp.tile([1, 512], F32, tag="rs", name=f"rs_{i}")
                        nc.vector.reciprocal_approx_fast(rs_r[:], rs0[:])
                        bc_sb = smallp.tile([DH, 512], F32, tag="bcs", name=f"bc_{i}")
                        nc.gpsimd.partition_broadcast(bc_sb[:], rs_r[:])
                        if h < 4:
                            nc.vector.tensor_mul(
                                zT[0:DH, h, tsl], ps_av[0:DH, :], bc_sb[:]
                            )
                        else:
                            ztmp = smallp.tile(
                                [DH, 512], F32R, tag="ztmp", name=f"zt_{i}"
                            )
                            nc.vector.tensor_mul(ztmp[:], ps_av[0:DH, :], bc_sb[:])
                            nc.sync.dma_start(zT[DH:P, h - 4, tsl], ztmp[:])

            # ---- phase D: output projection (partial over local heads) ----
            for sm in range(8):
                ps_o = psp.tile([P, 2, 512], F32, tag="mm2")
                for nt in range(2):
                    for hdc in range(4):
                        nc.tensor.matmul(
                            ps_o[:, nt, :],
                            lhsT=zT[:, hdc, sm * P : (sm + 1) * P],
                            rhs=Wo_sb[:, hdc, nt * 512 : (nt + 1) * 512],
                            start=(hdc == 0),
                            stop=(hdc == 3),
                        )
                ob = scrp.tile([P, 2, 512], F32, tag="scr")
                nc.vector.tensor_copy(ob[:], ps_o[:])
                nc.sync.dma_start(
                    zpart[sm * P : (sm + 1) * P, :],
                    ob[:].rearrange("p a b -> p (a b)"),
                )

    nc.compile()
    return nc


def _get_nc():
    if "nc" not in _CACHE:
        _CACHE["nc"] = _build()
    return _CACHE["nc"]


def _prep_core_inputs(x, Wqkv, Wo, scale_q, scale_k):
    """Host-side shard + layout prep. Returns list of 8 in_maps."""
    x = np.asarray(x, dtype=np.float32)
    Wqkv = np.asarray(Wqkv, dtype=np.float32)
    Wo = np.asarray(Wo, dtype=np.float32)
    scale_q = np.asarray(scale_q, dtype=np.float32)
    scale_k = np.asarray(scale_k, dtype=np.float32)

    # combined per-d_head scale folded into Q (applied after raw sum-sq)
    qs_vec = np.tile(scale_q * scale_k, NHL)          # [512]
    qs_dev = np.ascontiguousarray(qs_vec.reshape(4, P).T)  # [128,4]

    xt_all = []
    for b in range(4):
        xTb = _rne11(x[b].T)                           # [d, s]
        xt_all.append(
            np.ascontiguousarray(xTb.reshape(DC, P, S).transpose(1, 0, 2))
        )  # [128, 8, 1024]

    in_maps = []
    for c in range(8):
        b = c // 2
        hh = (c % 2) * NHL
        cols = slice(hh * DH, (hh + NHL) * DH)
        wq_c = Wqkv[:, 0 * D:1 * D][:, cols]           # [1024, 512]
        wk_c = Wqkv[:, 1 * D:2 * D][:, cols]
        wv_c = Wqkv[:, 2 * D:3 * D][:, cols]
        wqk_c = _rne11(np.concatenate([wq_c, wk_c], axis=1))  # [1024, 1024]
        # [p, ct, dc, n]: per-ct slices are contiguous 4KB-per-partition DMAs
        wqk_dev = np.ascontiguousarray(
            wqk_c.reshape(DC, P, 8, P).transpose(1, 2, 0, 3)
        )
        wv_dev = np.ascontiguousarray(
            _rne11(wv_c).reshape(DC, P, NHL * DH).transpose(1, 0, 2)
        )
        # Wo rows for local heads, arranged [128, 4, 1024]:
        # chunk k low half = head k, high half = head 4+k
        wo_loc = _rne11(Wo[(hh * DH):(hh + NHL) * DH, :])   # [512, 1024]
        wo_dev = np.empty((P, 4, D), dtype=np.float32)
        for k in range(4):
            wo_dev[0:DH, k, :] = wo_loc[k * DH:(k + 1) * DH, :]
            wo_dev[DH:P, k, :] = wo_loc[(4 + k) * DH:(5 + k) * DH, :]
        in_maps.append(
            {
                "xt": xt_all[b],
                "wqk": wqk_dev,
                "wv": wv_dev,
                "wo": np.ascontiguousarray(wo_dev),
                "qscale": qs_dev,
            }
        )
    return in_maps


def run(x, Wqkv, Wo, scale_q, scale_k, trace=False):
    nc = _get_nc()
    in_maps = _prep_core_inputs(x, Wqkv, Wo, scale_q, scale_k)
    res = run_bass_kernel_spmd(
        nc, in_maps[:N_CORES], core_ids=list(range(N_CORES)), trace=trace
    )
    out = np.empty((4, S, D), dtype=np.float32)
    for b in range(4):
        if N_CORES == 8:
            out[b] = res.results[2 * b]["zpart"] + res.results[2 * b + 1]["zpart"]
    return out, res


def kernel(x, Wqkv, Wo, scale_q, scale_k):
    out, _ = run(x, Wqkv, Wo, scale_q, scale_k, trace=False)
    return out
